# revision 1
# baseline (speedup 1.0000x reference)
"""AttentionPool Trainium2 Bass kernel.

Reference computation (per batch b):
    h      = tanh(x @ W1 + b1)          # [N, H*F]   (big matmul, bf16 on PE)
    scores = h @ W2 + b2                # [N, H]     (PE dot per head chunk)
    scores = where(mask, scores, -1e9)
    w      = softmax(scores, axis=N)    # per head
    pooled = w.T @ x                    # [H, D]
    y      = concat_h(pooled) @ Wout + bout   # [D]

Sharding: data-parallel over batch B=32 across 8 cores (4 batches/core).
Weights replicated. All matmuls in bf16 (fp32 PSUM accumulation); softmax
bias/scale paths in fp32. End-to-end error vs fp32 reference ~4e-3.

Layout notes (per core):
  - x is host-cast to bf16 and shipped twice: natural (pooling needs N on
    partitions) and host-pretransposed (the score matmul needs D on
    partitions) — both load as plain contiguous DMAs.
  - W1 host-prepped to [D, H*F], streamed as 4 independent column-quarter
    tiles so PE starts after ~1MB.
  - The score dot, pooling, and output projection use tile_position
    column-tiling (4 concurrent M=4 matmuls in separate PE column strips,
    strips recombined on DVE).
  - scores kept as [4(h), N] rows per batch; softmax reduces over the free
    dim; no max-shift needed (|scores| <= ||W2||_1 ~ 18; b2 cancels under
    softmax and is dropped); exp's accum_out gives the denominator for
    free. The softmax-weight transpose (n onto partitions) runs on PE via
    transpose-mode, pipelined with the pooling matmuls.
"""

import numpy as np
import ml_dtypes

import concourse.bass as bass
import concourse.mybir as mybir
import concourse.tile as tile
from concourse import bacc
from concourse.bass import ts
from concourse.bass_utils import run_bass_kernel_spmd
from concourse.masks import make_identity

BF16 = mybir.dt.bfloat16
FP32 = mybir.dt.float32
AFT = mybir.ActivationFunctionType

P = 128


class Cfg:
    def __init__(self, BL=4, N=2048, D=1024, H=4, F=512, TB=512):
        self.BL, self.N, self.D, self.H, self.F, self.TB = BL, N, D, H, F, TB
        self.HF = H * F
        self.KD = D // P          # k-chunks of D
        self.MC = self.HF // P    # hf-chunks
        self.NBLK = N // TB       # token blocks per batch
        self.NC = N // P          # n-chunks
        self.KOUT = (H * D) // P  # k-chunks of the output projection
        self.R = BL * H           # score rows per core
        assert self.MC % H == 0
        self.FC = self.MC // H    # f-chunks per head


def build_kernel(nc: bass.Bass, cfg: Cfg, reps: int = 1):
    c = cfg
    x_d = nc.dram_tensor("x", [c.BL, c.N, c.D], BF16, kind="ExternalInput").ap()
    xt_d = nc.dram_tensor("xt", [c.BL, c.KD, P, c.N], BF16, kind="ExternalInput").ap()
    m_d = nc.dram_tensor("m", [c.BL, c.H, c.N], BF16, kind="ExternalInput").ap()
    w1_d = nc.dram_tensor("w1", [c.KD, P, c.HF], BF16, kind="ExternalInput").ap()
    w2_d = nc.dram_tensor("w2", [c.MC, P, c.H], BF16, kind="ExternalInput").ap()
    b1_d = nc.dram_tensor("b1", [c.HF], FP32, kind="ExternalInput").ap()
    wout_d = nc.dram_tensor("wout", [c.KOUT, P, c.D], BF16, kind="ExternalInput").ap()
    bout_d = nc.dram_tensor("bout", [c.BL, c.D], FP32, kind="ExternalInput").ap()
    y_d = nc.dram_tensor("y", [c.BL, c.D], FP32, kind="ExternalOutput").ap()

    with tile.TileContext(nc) as tc:
        with (
            tc.tile_pool(name="const", bufs=1) as const,
            tc.tile_pool(name="xT", bufs=2) as xT_pool,
            tc.tile_pool(name="h", bufs=6) as h_pool,
            tc.tile_pool(name="xn", bufs=6) as xn_pool,
            tc.tile_pool(name="eT", bufs=2) as eT_pool,
            tc.tile_pool(name="sc", bufs=2) as sc_pool,
            tc.tile_pool(name="small", bufs=8) as small_pool,
            tc.tile_pool(name="sctmp", bufs=2) as sctmp_pool,
            tc.tile_pool(name="ysb", bufs=1) as ysb_pool,
            tc.tile_pool(name="hps", bufs=2, space="PSUM") as hps_pool,
            tc.tile_pool(name="scps", bufs=1, space="PSUM") as scps_pool,
            tc.tile_pool(name="tps", bufs=3, space="PSUM") as tps_pool,
            tc.tile_pool(name="plps", bufs=1, space="PSUM") as plps_pool,
        ):
            # ---- constants / weights ----
            # W1 as 4 independent column-quarter tiles: the first matmul
            # group only waits for quarter 0 (~1MB), the rest stream in
            # behind the first xT block
            QW = c.HF // 4
            w1q = []
            for q in range(4):
                t = const.tile([P, c.KD, QW], BF16, tag=f"w1q{q}")
                w1q.append(t)
            nc.sync.dma_start(
                w1q[0][:], w1_d[:, :, ts(0, QW)].rearrange("k p f -> p k f")
            )
            w2_sb = const.tile([P, c.MC, c.H], BF16)
            b1_sb = const.tile([P, c.MC], FP32)
            mask_sb = [
                const.tile([c.H, c.N], BF16, tag=f"mask{b}", name=f"mask{b}")
                for b in range(c.BL)
            ]
            bout_sb = const.tile([c.BL, c.D], FP32)
            idH_bf = const.tile([c.H, c.H], BF16)
            make_identity(nc, idH_bf[:])
            RP = c.BL * 32  # pooled rows: batch b at partition b*32 + h
            idR_f32 = const.tile([RP, RP], FP32)
            make_identity(nc, idR_f32[:])

            pooled_sb = const.tile([RP, c.D], FP32)
            nc.gpsimd.memset(pooled_sb[:], 0.0)
            poolT_sb = const.tile([P, c.KD, RP], BF16)
            wout_sb = const.tile([P, c.KOUT, c.D], BF16)

            for rep in range(reps):
              for b in range(c.BL):
                r0 = b * c.H
                if b == 1 and rep == 0:
                    # prefetch the output projection during the long middle
                    nc.sync.dma_start(
                        wout_sb[:], wout_d.rearrange("k p f -> p k f")
                    )
                sc_sb = sc_pool.tile([c.H, c.N], FP32, tag="scores")
                # ---- scores: h = tanh(x W1 + b1); s = h . W2 ----
                for blk in range(c.NBLK):
                    xT = xT_pool.tile([P, c.KD, c.TB], BF16)
                    nc.sync.dma_start(
                        xT[:], xt_d[b, :, :, ts(blk, c.TB)].rearrange("k p t -> p k t")
                    )
                    if b == 0 and blk == 0 and rep == 0:
                        for q in range(1, 4):
                            nc.sync.dma_start(
                                w1q[q][:],
                                w1_d[:, :, ts(q, QW)].rearrange("k p f -> p k f"),
                            )
                        nc.scalar.dma_start(
                            b1_sb[:], b1_d.rearrange("(c p) -> p c", p=P)
                        )
                        nc.scalar.dma_start(
                            w2_sb[:], w2_d.rearrange("c p h -> p c h")
                        )
                        for bb in range(c.BL):
                            nc.scalar.dma_start(mask_sb[bb][:], m_d[bb])
                        nc.scalar.dma_start(bout_sb[:], bout_d)
                    # score partials land in 4 PE column strips
                    # (tile_position col-tiling -> the 4 dots of a round
                    # run concurrently on HW); strips summed on DVE after
                    sc_ps = scps_pool.tile([P, c.TB], FP32)
                    NR = c.MC // 4
                    for rnd in range(NR):
                        h_tiles = []
                        for j in range(4):
                            mc = rnd * 4 + j
                            h_ps = hps_pool.tile([P, c.TB], FP32, tag="h_ps")
                            for dc in range(c.KD):
                                nc.tensor.matmul(
                                    h_ps[:],
                                    w1q[mc // (c.MC // 4)][:, dc, ts(mc % (c.MC // 4), P)],
                                    xT[:, dc, :],
                                    start=(dc == 0),
                                    stop=(dc == c.KD - 1),
                                )
                            h_sb = h_pool.tile([P, c.TB], BF16, tag="h_sb")
                            nc.scalar.activation(
                                h_sb[:], h_ps[:], AFT.Tanh,
                                bias=b1_sb[:, mc : mc + 1],
                            )
                            h_tiles.append(h_sb)
                        for j in range(4):
                            mc = rnd * 4 + j
                            nc.tensor.matmul(
                                sc_ps[32 * j : 32 * j + c.H, :],
                                w2_sb[:, mc, :],
                                h_tiles[j][:],
                                start=(rnd == 0),
                                stop=(rnd == NR - 1),
                                tile_position=(0, 32 * j),
                            )
                    # combine 4 strips + mask -> SBUF (DVE reads at
                    # most one PSUM operand per op, so chain via SBUF)
                    sctmp = sctmp_pool.tile([c.H, c.TB], FP32, tag="sctmp")
                    nc.vector.tensor_copy(sctmp[:], sc_ps[0 : c.H, :])
                    nc.vector.tensor_add(
                        sctmp[:], sctmp[:], sc_ps[32 : 32 + c.H, :]
                    )
                    nc.vector.tensor_add(
                        sctmp[:], sctmp[:], sc_ps[64 : 64 + c.H, :]
                    )
                    nc.vector.tensor_add(
                        sctmp[:], sctmp[:], sc_ps[96 : 96 + c.H, :]
                    )
                    nc.vector.tensor_add(
                        sc_sb[:, ts(blk, c.TB)],
                        sctmp[:],
                        mask_sb[b][:, ts(blk, c.TB)],
                    )
                # ---- softmax over N; no max-shift needed: |scores| <=
                # ||W2||_1 ~ 18 (|tanh|<1), well within fp32 exp range;
                # masked entries are -1e9 -> exp == 0. Two halves so the
                # e^T transposes can start after the first half.
                e_sb = sc_pool.tile([c.H, c.N], BF16, tag="e")
                zs = small_pool.tile([c.H, 2], FP32, tag="zs")
                # 3/4 + 1/4 split: the first span only depends on earlier
                # blocks, so e^T transposes + pooling start while the last
                # block's combine and exp-tail still run
                NA = 3 * c.N // 4
                nc.scalar.activation(
                    e_sb[:, 0:NA], sc_sb[:, 0:NA],
                    AFT.Exp, bias=0.0, accum_out=zs[:, 0:1],
                )
                nc.scalar.activation(
                    e_sb[:, NA : c.N], sc_sb[:, NA : c.N],
                    AFT.Exp, bias=0.0, accum_out=zs[:, 1:2],
                )
                zsum = small_pool.tile([c.H, 1], FP32, tag="zsum")
                nc.vector.tensor_add(zsum[:], zs[:, 0:1], zs[:, 1:2])
                rz = small_pool.tile([c.H, 1], FP32, tag="rz")
                nc.vector.reciprocal(rz[:], zsum[:])
                # ---- e^T via PE transpose, fused with pooling ----
                eT = eT_pool.tile([P, c.NC, c.H], BF16)
                pl_ps = plps_pool.tile([P, c.D], FP32, tag="plps")

                def emit_trans(cn):
                    tp = tps_pool.tile([P, c.R], BF16, tag="tps")
                    nc.tensor.transpose(
                        tp[:, : c.H], e_sb[:, ts(cn, P)], idH_bf[:]
                    )
                    if cn % 2 == 0:
                        nc.vector.tensor_copy(eT[:, cn, :], tp[:, : c.H])
                    else:
                        nc.scalar.copy(eT[:, cn, :], tp[:, : c.H])

                emit_trans(0)
                NS = min(4, c.NC)
                for cn in range(c.NC):
                    j = cn % NS
                    xn = xn_pool.tile([P, c.D], BF16)
                    nc.sync.dma_start(xn[:], x_d[b, ts(cn, P), :])
                    if cn + 1 < c.NC:
                        emit_trans(cn + 1)
                    for half in range(max(1, c.D // 512)):
                        wd = min(512, c.D)
                        nc.tensor.matmul(
                            pl_ps[32 * j : 32 * j + c.H, ts(half, wd)],
                            eT[:, cn, :],
                            xn[:, ts(half, wd)],
                            start=(cn < NS),
                            stop=(cn >= c.NC - NS),
                            tile_position=(0, 32 * j),
                        )
                pltmp = sctmp_pool.tile([c.H, c.D], FP32, tag="pltmp")
                HD = c.D // 2 if c.D >= 256 else c.D
                for hf in range(c.D // HD):
                    sl = ts(hf, HD)
                    nc.vector.tensor_copy(pltmp[:, sl], pl_ps[0 : c.H, sl])
                    for j in range(1, NS):
                        nc.vector.tensor_add(
                            pltmp[:, sl], pltmp[:, sl],
                            pl_ps[32 * j : 32 * j + c.H, sl],
                        )
                    nc.vector.tensor_scalar_mul(
                        pooled_sb[b * 32 : b * 32 + c.H, sl], pltmp[:, sl], rz[:]
                    )

              # ---- pooled^T and output projection (col-tiled over heads) ----
              fin_ps = plps_pool.tile([P, c.D], FP32, tag="plps")
              nhalf = max(1, c.D // 512)
              w = c.D // nhalf
              for dc in range(c.KD):
                tp2 = tps_pool.tile([P, RP], FP32, tag="tps")
                nc.tensor.transpose(tp2[:], pooled_sb[:, ts(dc, P)], idR_f32[:])
                nc.vector.tensor_copy(poolT_sb[:, dc, :], tp2[:])
                for hd in range(c.H):
                    k = hd * c.KD + dc
                    j = hd % 2
                    lhsT = poolT_sb[:, dc, :].rearrange(
                        "p (b j) -> p j b", j=32
                    )[:, hd, :]
                    for half in range(nhalf):
                        nc.tensor.matmul(
                            fin_ps[32 * j : 32 * j + c.BL, ts(half, w)],
                            lhsT,
                            wout_sb[:, k, ts(half, w)],
                            start=(dc == 0 and hd < 2),
                            stop=(dc == c.KD - 1 and hd >= c.H - 2),
                            tile_position=(0, 32 * j),
                        )
              y_sb = ysb_pool.tile([c.BL, c.D], FP32)
              nc.vector.tensor_copy(y_sb[:], fin_ps[0 : c.BL, :])
              nc.vector.tensor_add(y_sb[:], y_sb[:], fin_ps[32 : 32 + c.BL, :])
              nc.vector.tensor_add(y_sb[:], y_sb[:], bout_sb[:])
              nc.sync.dma_start(y_d[:], y_sb[:])
    return nc


def make_in_maps(x, valid_mask, W1, b1, W2, b2, Wout, bout, n_cores, cfg):
    """Host-side prep: shard over batch, cast/layout weights."""
    c = cfg
    bf16 = ml_dtypes.bfloat16
    x_bf = np.ascontiguousarray(x.astype(bf16))
    # additive mask with b2 baked in, rows = b*H + h
    madd = np.where(valid_mask, np.float32(0), np.float32(-1e9))  # [B, N]
    w1_l = np.ascontiguousarray(
        W1.transpose(1, 0, 2).reshape(c.KD, P, c.HF).astype(bf16)
    )
    w2f = W2.reshape(c.HF).astype(np.float32)
    w2_l = np.zeros((c.MC, P, c.H), np.float32)
    for mc in range(c.MC):
        w2_l[mc, :, mc // c.FC] = w2f[mc * P : (mc + 1) * P]
    w2_l = np.ascontiguousarray(w2_l.astype(bf16))
    b1_l = np.ascontiguousarray(b1.reshape(c.HF).astype(np.float32))
    wout_l = np.ascontiguousarray(Wout.reshape(c.KOUT, P, c.D).astype(bf16))
    bout_l = np.ascontiguousarray(
        np.broadcast_to(bout.astype(np.float32), (c.BL, c.D))
    )
    xt_all = np.ascontiguousarray(x_bf.transpose(0, 2, 1)).reshape(
        x_bf.shape[0], c.KD, P, c.N
    )
    # b2 is a per-row constant under the softmax -> it cancels; drop it.
    madd_bf = np.broadcast_to(
        madd.astype(bf16)[:, None, :], (madd.shape[0], c.H, c.N)
    )
    in_maps = []
    for core in range(n_cores):
        b0 = core * c.BL
        in_maps.append(
            {
                "x": np.ascontiguousarray(x_bf[b0 : b0 + c.BL]),
                "xt": np.ascontiguousarray(xt_all[b0 : b0 + c.BL]),
                "m": np.ascontiguousarray(madd_bf[b0 : b0 + c.BL]),
                "w1": w1_l,
                "w2": w2_l,
                "b1": b1_l,
                "wout": wout_l,
                "bout": bout_l,
            }
        )
    return in_maps


_cached = {}
last_results = None


def kernel(x, valid_mask, W1, b1, W2, b2, Wout, bout, trace=False):
    global last_results
    x, valid_mask, W1, b1, W2, b2, Wout, bout = (
        np.asarray(a)
        for a in (x, valid_mask, W1, b1, W2, b2, Wout, bout)
    )
    B = x.shape[0]
    n_cores = 8
    cfg = Cfg(BL=B // n_cores)
    key = (B, trace)
    if "nc" not in _cached:
        nc = bacc.Bacc("TRN2", target_bir_lowering=False, debug=False)
        build_kernel(nc, cfg)
        nc.compile()
        _cached["nc"] = nc
    in_maps = make_in_maps(x, valid_mask, W1, b1, W2, b2, Wout, bout, n_cores, cfg)
    res = run_bass_kernel_spmd(
        _cached["nc"], in_maps, core_ids=list(range(n_cores)), trace=trace
    )
    last_results = res
    y = np.concatenate([res.results[i]["y"] for i in range(n_cores)], axis=0)
    return y.astype(np.float32)



# revision 17
# speedup vs baseline: 1.8815x; 1.8815x over previous
"""AttentionPool Trainium2 Bass kernel (v2: valid-token compaction).

Reference computation (per batch b):
    h      = tanh(x @ W1 + b1)          # [N, H*F]   (big matmul, bf16 on PE)
    scores = h @ W2 + b2                # [N, H]
    scores = where(mask, scores, -1e9)
    w      = softmax(scores, axis=N)    # per head
    pooled = w.T @ x                    # [H, D]
    y      = concat_h(pooled) @ Wout + bout   # [D]

Key observations exploited here:
  - Invalid tokens get softmax weight 0, so they contribute nothing to the
    output. The host compacts each batch's valid tokens (~1024 of 2048 at
    p=0.5) into a contiguous padded buffer of NV tokens (NV = max valid
    count rounded up to 128, chosen at runtime). The big x@W1 matmul then
    runs on NV instead of N tokens (~1.8x less PE work).
  - Padding slots carry x=0, so they pollute neither the pooling sum
    (e_pad * 0 = 0) nor - after the fix below - the softmax denominator.
    Instead of an additive -1e9 mask, the denominator Z is computed as a
    validity-weighted reduction Z = sum_t v_t * e_t on the PE (v=1 valid,
    0 pad), so no mask tensor exists at all. b2 cancels under softmax and
    is dropped.
  - The score dot (h . W2) is shaped to produce [128tok, H] tiles
    (free dim = H = 4) accumulated over the 16 f-chunks, instead of
    [H, tok] rows: tiny output rows stream off the PE in a few cycles per
    instruction, and the result lands token-major, which is exactly the
    lhsT layout pooling needs for exp(scores) - no e transposes.

Sharding: data-parallel over batch B=32 across 8 cores (4 batches/core).
Weights replicated. Matmuls in bf16 (fp32 PSUM accumulation); softmax in
fp32 on the Act engine (exp_and_others act table holds both tanh and exp,
so no table reloads). No max-shift needed: |scores| <= ||W2||_1 ~ 18.

Pipelining (keeps the PE dense, which also keeps its p-state ramped):
  - per f-chunk mc: the 8 k-matmuls of chunk mc are issued before the
    score dots of chunk mc-1, so dots never stall on the tanh.
  - a block's pooling + Z matmuls are injected into the NEXT block's
    instruction stream (after its first h-matmul group), when exp(scores)
    has long finished.
  - per-batch finish (1/Z, pooled scale, pooled^T) rides the same queue;
    the output projection consumes pooled^T of all 4 batches at the end.
"""

import numpy as np
import ml_dtypes

import concourse.bass as bass
import concourse.mybir as mybir
import concourse.tile as tile
from concourse import bacc
from concourse.bass import ts
from concourse.bass_utils import run_bass_kernel_spmd
from concourse.masks import make_identity

BF16 = mybir.dt.bfloat16
FP32 = mybir.dt.float32
AFT = mybir.ActivationFunctionType

P = 128


class Cfg:
    def __init__(self, BL=4, NV=1152, D=1024, H=4, F=512):
        self.BL, self.NV, self.D, self.H, self.F = BL, NV, D, H, F
        self.HF = H * F
        self.KD = D // P          # k-chunks of D
        self.MC = self.HF // P    # f-chunks of H*F
        self.NCV = NV // P        # token chunks
        self.KOUT = (H * D) // P  # k-chunks of the output projection
        assert NV % P == 0
        # token blocks: 512-wide (one PSUM bank of h) + remainder
        self.blocks = [512] * (NV // 512)
        if NV % 512:
            self.blocks.append(NV % 512)


def choose_nv(valid_mask: np.ndarray) -> int:
    """Smallest multiple of 128 covering every batch's valid-token count."""
    cnt = int(np.asarray(valid_mask).sum(axis=1).max())
    return max(P, -(-cnt // P) * P)


def build_kernel(nc: bass.Bass, cfg: Cfg, reps: int = 1):
    c = cfg
    HD = c.D // 2  # PSUM-bank half of D
    xt_d = nc.dram_tensor("xt", [c.BL, c.KD, P, c.NV], BF16, kind="ExternalInput").ap()
    xn_d = nc.dram_tensor("xn", [c.BL, c.NV, c.D], BF16, kind="ExternalInput").ap()
    v_d = nc.dram_tensor("v", [c.BL, P, c.NCV], BF16, kind="ExternalInput").ap()
    w1_d = nc.dram_tensor("w1", [c.KD, P, c.HF], BF16, kind="ExternalInput").ap()
    w2_d = nc.dram_tensor("w2", [c.MC, P, c.H], BF16, kind="ExternalInput").ap()
    b1_d = nc.dram_tensor("b1", [c.HF], FP32, kind="ExternalInput").ap()
    wout_d = nc.dram_tensor("wout", [c.KOUT, P, c.D], BF16, kind="ExternalInput").ap()
    bout_d = nc.dram_tensor("bout", [c.BL, c.D], FP32, kind="ExternalInput").ap()
    y_d = nc.dram_tensor("y", [c.BL, c.D], FP32, kind="ExternalOutput").ap()

    with tile.TileContext(nc) as tc:
        with (
            tc.tile_pool(name="const", bufs=1) as const,
            tc.tile_pool(name="xT", bufs=2) as xT_pool,
            tc.tile_pool(name="h", bufs=4) as h_pool,
            tc.tile_pool(name="xn", bufs=3) as xn_pool,
            tc.tile_pool(name="e", bufs=3) as e_pool,
            tc.tile_pool(name="small", bufs=8) as small_pool,
            tc.tile_pool(name="ysb", bufs=1) as ysb_pool,
            tc.tile_pool(name="hps", bufs=3, space="PSUM") as hps_pool,
            tc.tile_pool(name="scps", bufs=2, space="PSUM") as scps_pool,
            tc.tile_pool(name="plps", bufs=1, space="PSUM") as plps_pool,
            tc.tile_pool(name="zps", bufs=1, space="PSUM") as zps_pool,
        ):
            # ---- constants / weights ----
            # W1 as 4 column-quarter tiles: the first matmul group only
            # waits for quarter 0 (~1MB); the rest stream in behind it.
            QW = c.HF // 4
            QMC = c.MC // 4  # f-chunks per quarter
            w1q = [
                const.tile([P, c.KD, QW], BF16, tag=f"w1q{q}", name=f"w1q{q}")
                for q in range(4)
            ]
            nc.sync.dma_start(
                w1q[0][:], w1_d[:, :, ts(0, QW)].rearrange("k p f -> p k f")
            )
            w2_sb = const.tile([P, c.MC, c.H], BF16)
            b1_sb = const.tile([P, c.MC], FP32)
            v_sb = const.tile([P, c.BL, c.NCV], BF16)
            bout_sb = const.tile([c.BL, c.D], FP32)
            # pooled rows at r = b*32 + h (32-aligned per batch for DVE)
            idR = const.tile([P, P], FP32)
            make_identity(nc, idR[:])
            pooled_sb = const.tile([P, c.D], FP32)
            nc.gpsimd.memset(pooled_sb[:], 0.0)
            poolT_sb = const.tile([P, c.KD, P], BF16)
            wout_sb = const.tile([P, c.KOUT, c.D], BF16)

            pending = []  # deferred (other-block) op emitters

            def flush():
                while pending:
                    pending.pop(0)()

            for b in range(c.BL):
                # pl/z PSUM tiles are bufs=1: allocate only after the
                # previous batch's deferred consumers have been emitted
                # (first flush of this batch), so buffer-reuse tracking
                # sees ops in order.
                pl_ps = z_ps = None
                t0 = 0
                for bi, TB in enumerate(c.blocks):
                    subs = TB // P
                    cn0 = t0 // P
                    xT = xT_pool.tile([P, c.KD, 512], BF16)
                    nc.sync.dma_start(
                        xT[:, :, 0:TB],
                        xt_d[b, :, :, t0 : t0 + TB].rearrange("k p t -> p k t"),
                    )
                    xnt = xn_pool.tile([P, 4, c.D], BF16)
                    nc.sync.dma_start(
                        xnt[:, 0:subs, :],
                        xn_d[b, t0 : t0 + TB, :].rearrange("(s p) d -> p s d", p=P),
                    )
                    if b == 0 and bi == 0:
                        for q in range(1, 4):
                            nc.sync.dma_start(
                                w1q[q][:],
                                w1_d[:, :, ts(q, QW)].rearrange("k p f -> p k f"),
                            )
                        nc.scalar.dma_start(
                            b1_sb[:], b1_d.rearrange("(c p) -> p c", p=P)
                        )
                        nc.scalar.dma_start(
                            w2_sb[:], w2_d.rearrange("c p h -> p c h")
                        )
                        nc.scalar.dma_start(
                            v_sb[:], v_d.rearrange("b p c -> p b c")
                        )
                        nc.scalar.dma_start(bout_sb[:], bout_d)
                    if b == 1 and bi == 0:
                        # prefetch the output projection during the middle
                        KQ = c.KOUT // 4
                        for q in range(4):
                            nc.sync.dma_start(
                                wout_sb[:, ts(q, KQ), :],
                                wout_d[ts(q, KQ)].rearrange("k p f -> p k f"),
                            )
                    # 128-token compute sub-blocks: one sc accumulation
                    # group (PSUM zero region) live at a time.
                    for s in range(subs):
                        cn = cn0 + s
                        sc_ps = scps_pool.tile([P, 512], FP32, tag="sc_ps")
                        prevq = []
                        for mc in range(c.MC):
                            h_ps = hps_pool.tile([P, 512], FP32, tag="h_ps")
                            wcol = (mc % QMC) * P
                            for dc in range(c.KD):
                                nc.tensor.matmul(
                                    h_ps[:, 0:P],
                                    w1q[mc // QMC][:, dc, wcol : wcol + P],
                                    xT[:, dc, s * P : (s + 1) * P],
                                    start=(dc == 0),
                                    stop=(dc == c.KD - 1),
                                )
                            h_sb = h_pool.tile([P, P], BF16, tag="h_sb")
                            nc.scalar.activation(
                                h_sb[:], h_ps[:, 0:P], AFT.Tanh,
                                bias=b1_sb[:, mc : mc + 1],
                            )
                            if mc == 2:
                                flush()

                            def dot(mc, h_sb):
                                nc.tensor.matmul(
                                    sc_ps[:, 0 : c.H],
                                    h_sb[:],
                                    w2_sb[:, mc, :],
                                    start=(mc == 0),
                                    stop=(mc == c.MC - 1),
                                )

                            prevq.append((mc, h_sb))
                            if len(prevq) > 2:
                                dot(*prevq.pop(0))
                        while prevq:
                            dot(*prevq.pop(0))
                        e_blk = e_pool.tile([P, c.H], BF16, tag="e_blk")
                        nc.scalar.activation(
                            e_blk[:], sc_ps[:, 0 : c.H], AFT.Exp, bias=0.0,
                        )

                        if pl_ps is None:
                            pl_ps = plps_pool.tile([P, c.D], FP32, tag="plps")
                            z_ps = zps_pool.tile([P, 512], FP32, tag="zps")

                        def mk_pool(b, s, cn, e_blk, xnt, pl_ps, z_ps):
                            def go():
                                nc.tensor.matmul(
                                    z_ps[0 : c.H, 0:1],
                                    e_blk[:],
                                    v_sb[:, b, cn : cn + 1],
                                    start=(cn == 0),
                                    stop=(cn == c.NCV - 1),
                                )
                                for hf in range(2):
                                    nc.tensor.matmul(
                                        pl_ps[0 : c.H, ts(hf, HD)],
                                        e_blk[:],
                                        xnt[:, s, ts(hf, HD)],
                                        start=(cn == 0),
                                        stop=(cn == c.NCV - 1),
                                    )
                            return go

                        pending.append(mk_pool(b, s, cn, e_blk, xnt, pl_ps, z_ps))
                    t0 += TB

                def mk_finish(b, pl_ps, z_ps):
                    def go():
                        z_sb = small_pool.tile([c.H, 1], FP32, tag="z_sb")
                        nc.vector.tensor_copy(z_sb[:], z_ps[0 : c.H, 0:1])
                        rz = small_pool.tile([c.H, 1], FP32, tag="rz")
                        nc.vector.reciprocal(rz[:], z_sb[:])
                        r0 = b * 32
                        for hf in range(2):
                            nc.vector.tensor_scalar_mul(
                                pooled_sb[r0 : r0 + c.H, ts(hf, HD)],
                                pl_ps[0 : c.H, ts(hf, HD)],
                                rz[:],
                            )
                    return go

                pending.append(mk_finish(b, pl_ps, z_ps))

            flush()
            # ---- pooled^T, then output projection y = pooled @ Wout + bout ----
            # tpp reuses sc-pool buffers: sc groups are long closed by now.
            for grp in range(2):
                tpp = scps_pool.tile([P, 512], FP32, tag="sc_ps", name="tpp")
                for i in range(4):
                    dc = grp * 4 + i
                    nc.tensor.transpose(
                        tpp[:, i * P : (i + 1) * P],
                        pooled_sb[:, ts(dc, P)],
                        idR[:],
                    )
                nc.vector.tensor_copy(
                    poolT_sb[:, grp * 4 : (grp + 1) * 4, :],
                    tpp[:].rearrange("p (k r) -> p k r", r=P),
                )
            fin_ps = plps_pool.tile([P, c.D], FP32, tag="plps")
            for dc in range(c.KD):
                bcols = poolT_sb[:, dc, :].rearrange("p (b j) -> p j b", j=32)
                for hd in range(c.H):
                    k = hd * c.KD + dc
                    for hf in range(2):
                        nc.tensor.matmul(
                            fin_ps[0 : c.BL, ts(hf, HD)],
                            bcols[:, hd, :],
                            wout_sb[:, k, ts(hf, HD)],
                            start=(dc == 0 and hd == 0),
                            stop=(dc == c.KD - 1 and hd == c.H - 1),
                        )
            y_sb = ysb_pool.tile([c.BL, c.D], FP32)
            for hf in range(2):
                nc.vector.tensor_add(
                    y_sb[:, ts(hf, HD)], fin_ps[0 : c.BL, ts(hf, HD)],
                    bout_sb[:, ts(hf, HD)],
                )
            nc.sync.dma_start(y_d[:], y_sb[:])
    return nc


def make_in_maps(x, valid_mask, W1, b1, W2, b2, Wout, bout, n_cores, cfg):
    """Host-side prep: compact valid tokens, shard over batch, cast/layout."""
    c = cfg
    bf16 = ml_dtypes.bfloat16
    B = x.shape[0]
    w1_l = np.ascontiguousarray(
        W1.transpose(1, 0, 2).reshape(c.KD, P, c.HF).astype(bf16)
    )
    w2f = W2.reshape(c.HF).astype(np.float32)
    w2_l = np.zeros((c.MC, P, c.H), np.float32)
    FC = c.MC // c.H  # f-chunks per head
    for mc in range(c.MC):
        w2_l[mc, :, mc // FC] = w2f[mc * P : (mc + 1) * P]
    w2_l = np.ascontiguousarray(w2_l.astype(bf16))
    b1_l = np.ascontiguousarray(b1.reshape(c.HF).astype(np.float32))
    wout_l = np.ascontiguousarray(Wout.reshape(c.KOUT, P, c.D).astype(bf16))
    bout_l = np.ascontiguousarray(
        np.broadcast_to(bout.astype(np.float32), (c.BL, c.D))
    )
    # b2 is a per-row constant under the softmax -> it cancels; drop it.
    xc = np.zeros((B, c.NV, c.D), bf16)
    v = np.zeros((B, c.NV), np.float32)
    for gb in range(B):
        idx = np.flatnonzero(valid_mask[gb])
        xc[gb, : len(idx)] = x[gb, idx].astype(bf16)
        v[gb, : len(idx)] = 1.0
    v_l = np.ascontiguousarray(
        v.reshape(B, c.NCV, P).transpose(0, 2, 1).astype(bf16)
    )
    xt_all = np.ascontiguousarray(xc.transpose(0, 2, 1)).reshape(
        B, c.KD, P, c.NV
    )
    in_maps = []
    for core in range(n_cores):
        b0 = core * c.BL
        in_maps.append(
            {
                "xt": xt_all[b0 : b0 + c.BL],
                "xn": xc[b0 : b0 + c.BL],
                "v": v_l[b0 : b0 + c.BL],
                "w1": w1_l,
                "w2": w2_l,
                "b1": b1_l,
                "wout": wout_l,
                "bout": bout_l,
            }
        )
    return in_maps


_cached = {}
last_results = None


def kernel(x, valid_mask, W1, b1, W2, b2, Wout, bout, trace=False):
    global last_results
    x, valid_mask, W1, b1, W2, b2, Wout, bout = (
        np.asarray(a)
        for a in (x, valid_mask, W1, b1, W2, b2, Wout, bout)
    )
    B = x.shape[0]
    n_cores = 8
    NV = choose_nv(valid_mask)
    cfg = Cfg(BL=B // n_cores, NV=NV)
    key = (B, NV)
    if key not in _cached:
        nc = bacc.Bacc("TRN2", target_bir_lowering=False, debug=False)
        build_kernel(nc, cfg)
        nc.compile()
        _cached[key] = nc
    in_maps = make_in_maps(x, valid_mask, W1, b1, W2, b2, Wout, bout, n_cores, cfg)
    res = run_bass_kernel_spmd(
        _cached[key], in_maps, core_ids=list(range(n_cores)), trace=trace
    )
    last_results = res
    y = np.concatenate([res.results[i]["y"] for i in range(n_cores)], axis=0)
    return y.astype(np.float32)


# revision 51
# speedup vs baseline: 94.4153x; 50.1818x over previous
"""AttentionPool Trainium2 Bass kernel (v2: valid-token compaction).

Reference computation (per batch b):
    h      = tanh(x @ W1 + b1)          # [N, H*F]   (big matmul, bf16 on PE)
    scores = h @ W2 + b2                # [N, H]
    scores = where(mask, scores, -1e9)
    w      = softmax(scores, axis=N)    # per head
    pooled = w.T @ x                    # [H, D]
    y      = concat_h(pooled) @ Wout + bout   # [D]

Key observations exploited here:
  - Invalid tokens get softmax weight 0, so they contribute nothing to the
    output. The host compacts each batch's valid tokens (~1024 of 2048 at
    p=0.5) into a contiguous padded buffer of NV tokens (NV = max valid
    count rounded up to 128, chosen at runtime). The big x@W1 matmul then
    runs on NV instead of N tokens (~1.8x less PE work).
  - Padding slots carry x=0, so they pollute neither the pooling sum
    (e_pad * 0 = 0) nor - after the fix below - the softmax denominator.
    Instead of an additive -1e9 mask, the denominator Z is computed as a
    validity-weighted reduction Z = sum_t v_t * e_t on the PE (v=1 valid,
    0 pad), so no mask tensor exists at all. b2 cancels under softmax and
    is dropped.
  - The score dot (h . W2) is shaped to produce [128tok, H] tiles
    (free dim = H = 4) accumulated over the 16 f-chunks, instead of
    [H, tok] rows: tiny output rows stream off the PE in a few cycles per
    instruction, and the result lands token-major, which is exactly the
    lhsT layout pooling needs for exp(scores) - no e transposes.

Sharding: data-parallel over batch B=32 across 8 cores (4 batches/core).
Weights replicated. Matmuls in bf16 (fp32 PSUM accumulation); softmax in
fp32 on the Act engine (exp_and_others act table holds both tanh and exp,
so no table reloads). No max-shift needed: |scores| <= ||W2||_1 ~ 18.

Pipelining (keeps the PE dense, which also keeps its p-state ramped):
  - per f-chunk mc: the 8 k-matmuls of chunk mc are issued before the
    score dots of chunk mc-1, so dots never stall on the tanh.
  - a block's pooling + Z matmuls are injected into the NEXT block's
    instruction stream (after its first h-matmul group), when exp(scores)
    has long finished.
  - per-batch finish (1/Z, pooled scale, pooled^T) rides the same queue;
    the output projection consumes pooled^T of all 4 batches at the end.
"""

import numpy as np
import ml_dtypes

import concourse.bass as bass
import concourse.mybir as mybir
import concourse.tile as tile
from concourse import bacc
from concourse.bass import ts
from concourse.bass_utils import run_bass_kernel_spmd
from concourse.masks import make_identity

BF16 = mybir.dt.bfloat16
FP32 = mybir.dt.float32
AFT = mybir.ActivationFunctionType

P = 128


class Cfg:
    def __init__(self, BL=4, NVS=(1152,) * 4, D=1024, H=4, F=512):
        # NVS: per-batch-slot padded valid-token count (batches are
        # assigned to slots sorted by count, so later slots can be shorter)
        self.BL, self.NVS, self.D, self.H, self.F = BL, tuple(NVS), D, H, F
        assert len(self.NVS) == BL
        self.NV = max(self.NVS)   # shipped tensor size
        self.HF = H * F
        self.KD = D // P          # k-chunks of D
        self.MC = self.HF // P    # f-chunks of H*F
        self.NCV = -(-self.NV // P)  # token chunks (max slot, ceil)
        self.KOUT = (H * D) // P  # k-chunks of the output projection

    def blocks(self, b):
        """DMA token blocks for batch slot b: 512-wide + remainder."""
        nv = self.NVS[b]
        out = [512] * (nv // 512)
        if nv % 512:
            out.append(nv % 512)
        return out


def choose_slots(valid_mask: np.ndarray, n_cores=8, BL=4):
    """Sort batches by valid count; slot i takes ranks [i*n_cores,
    (i+1)*n_cores) so each slot's NV covers its 8 batches exactly
    (partial trailing token chunks are fine - tokens ride free/K dims).
    Returns (NVS, order) with order[i*n_cores + c] = original batch
    index processed by core c in slot i."""
    cnt = np.asarray(valid_mask).sum(axis=1)
    order = np.argsort(-cnt, kind="stable")
    NVS = []
    for i in range(BL):
        grp = cnt[order[i * n_cores : (i + 1) * n_cores]]
        NVS.append(max(2, int(grp.max())))
    return tuple(NVS), order


def build_kernel(nc: bass.Bass, cfg: Cfg, reps: int = 1):
    c = cfg
    HD = c.D // 2  # PSUM-bank half of D
    xt_d = nc.dram_tensor("xt", [c.BL, c.KD, P, c.NV], BF16, kind="ExternalInput").ap()
    xn_d = nc.dram_tensor("xn", [c.BL, c.NV, c.D], BF16, kind="ExternalInput").ap()
    v_d = nc.dram_tensor("v", [c.BL, P, c.NCV], BF16, kind="ExternalInput").ap()
    w1_d = nc.dram_tensor("w1", [c.KD, P, c.HF], BF16, kind="ExternalInput").ap()
    w2_d = nc.dram_tensor("w2", [c.MC, P, c.H], BF16, kind="ExternalInput").ap()
    b1_d = nc.dram_tensor("b1", [c.HF], FP32, kind="ExternalInput").ap()
    wout_d = nc.dram_tensor("wout", [c.KOUT, P, c.D], BF16, kind="ExternalInput").ap()
    bout_d = nc.dram_tensor("bout", [P, c.KD * c.BL], FP32, kind="ExternalInput").ap()
    y_d = nc.dram_tensor("y", [c.BL, c.D], FP32, kind="ExternalOutput").ap()

    with tile.TileContext(nc) as tc:
        with (
            tc.tile_pool(name="const", bufs=1) as const,
            tc.tile_pool(name="xT", bufs=3) as xT_pool,
            tc.tile_pool(name="h", bufs=4) as h_pool,
            tc.tile_pool(name="xn", bufs=3) as xn_pool,
            tc.tile_pool(name="e", bufs=6) as e_pool,
            tc.tile_pool(name="small", bufs=8) as small_pool,
            tc.tile_pool(name="ysb", bufs=1) as ysb_pool,
            tc.tile_pool(name="hps", bufs=3, space="PSUM") as hps_pool,
            tc.tile_pool(name="scps", bufs=2, space="PSUM") as scps_pool,
            tc.tile_pool(name="plps", bufs=1, space="PSUM") as plps_pool,
            tc.tile_pool(name="zps", bufs=1, space="PSUM") as zps_pool,
        ):
            # ---- constants / weights ----
            # W1 as 4 column-quarter tiles: the first matmul group only
            # waits for quarter 0 (~1MB); the rest stream in behind it.
            QW = c.HF // 4
            QMC = c.MC // 4  # f-chunks per quarter
            w1q = [
                const.tile([P, c.KD, QW], BF16, tag=f"w1q{q}", name=f"w1q{q}")
                for q in range(4)
            ]

            def w1_piece(q, half):
                # 256-col half-quarter: 512B innermost -> full DMA rate
                f0 = q * QW + half * 256
                nc.sync.dma_start(
                    w1q[q][:, :, half * 256 : (half + 1) * 256],
                    w1_d[:, :, f0 : f0 + 256].rearrange("k p f -> p k f"),
                )

            w1_piece(0, 0)
            w2_sb = const.tile([P, c.MC, c.H], BF16)
            b1_sb = const.tile([P, c.MC], FP32)
            v_sb = const.tile([P, c.BL, c.NCV], BF16)
            bout_sb = const.tile([P, c.KD * c.BL], FP32)
            # pooled rows at r = b*32 + h (32-aligned per batch for DVE)
            idR = const.tile([P, P], FP32)
            make_identity(nc, idR[:])
            pooled_sb = const.tile([P, c.D], FP32)
            nc.gpsimd.memset(pooled_sb[:], 0.0)
            poolT_sb = const.tile([P, c.KD, P], BF16)
            wout_sb = const.tile([P, c.KOUT, c.D], BF16)

            pending = []  # deferred (other-block) op emitters

            def flush(limit=0):
                # keep `limit` closures queued: pool matmuls lag their
                # sub-block by a few positions so early ones never stall
                # the PE on the xn DMA
                while len(pending) > limit:
                    pending.pop(0)()

            # flat chunk schedule across batches; xT DMAs prefetch 2 ahead
            sched = []
            for b in range(c.BL):
                t0 = 0
                for bi, TB in enumerate(c.blocks(b)):
                    sched.append((b, bi, t0, TB))
                    t0 += TB
            xts = {}

            def ensure_xt(k):
                if k >= len(sched) or k in xts:
                    return
                b, bi, t0, TB = sched[k]
                xT = xT_pool.tile([P, c.KD, 512], BF16, name=f"xT{k % 3}")
                if k == 0 and TB == 512:
                    # split the first xT so the very first sub-pair
                    # only waits for half the tile
                    nc.sync.dma_start(
                        xT[:, :, 0:256],
                        xt_d[b, :, :, 0:256].rearrange("k p t -> p k t"),
                    )
                    # small weights on the scalar queue (needed early)
                    nc.scalar.dma_start(
                        b1_sb[:], b1_d.rearrange("(c p) -> p c", p=P)
                    )
                    nc.scalar.dma_start(
                        w2_sb[:], w2_d.rearrange("c p h -> p c h")
                    )
                    nc.scalar.dma_start(
                        v_sb[:], v_d.rearrange("b p c -> p b c")
                    )
                    nc.scalar.dma_start(bout_sb[:], bout_d)
                    # rest of W1 streams in half-quarters, paced to the
                    # interleaved first sub-pair's f-chunk consumption
                    w1_piece(0, 1)
                    for q in range(1, 4):
                        w1_piece(q, 0)
                        w1_piece(q, 1)
                    nc.sync.dma_start(
                        xT[:, :, 256:512],
                        xt_d[b, :, :, 256:512].rearrange("k p t -> p k t"),
                    )
                else:
                    nc.sync.dma_start(
                        xT[:, :, 0:TB],
                        xt_d[b, :, :, t0 : t0 + TB].rearrange("k p t -> p k t"),
                    )
                xts[k] = xT

            kprev = -1
            for b in range(c.BL):
                # pl/z PSUM tiles are bufs=1: allocate only after the
                # previous batch's deferred consumers have been emitted
                # (first flush of this batch), so buffer-reuse tracking
                # sees ops in order.
                pl_ps = z_ps = None
                NCVb = -(-c.NVS[b] // P)
                for bi, TB in enumerate(c.blocks(b)):
                    kprev += 1
                    k = kprev
                    _, _, t0, _ = sched[k]
                    cn0 = t0 // P
                    first = k == 0 and TB == 512
                    ensure_xt(k)
                    ensure_xt(k + 1)
                    ensure_xt(k + 2)
                    xT = xts.pop(k)
                    xnt = xn_pool.tile([P, 4, c.D], BF16)
                    fullt = (TB // P) * P  # whole 128-token chunks
                    if fullt:
                        nc.sync.dma_start(
                            xnt[:, 0 : TB // P, :],
                            xn_d[b, t0 : t0 + fullt, :].rearrange(
                                "(s p) d -> p s d", p=P
                            ),
                        )
                    if TB % P:
                        nc.sync.dma_start(
                            xnt[0 : TB % P, TB // P, :],
                            xn_d[b, t0 + fullt : t0 + TB, :],
                        )
                    if b == 1 and bi == 0:
                        # prefetch the output projection during the middle
                        KQ = c.KOUT // 4
                        for q in range(4):
                            nc.sync.dma_start(
                                wout_sb[:, ts(q, KQ), :],
                                wout_d[ts(q, KQ)].rearrange("k p f -> p k f"),
                            )
                    # 128-token compute sub-blocks; the first chunk of the
                    # kernel runs subs 0+1 interleaved so the W1 stream
                    # keeps up with the PE's f-chunk consumption.
                    subs = -(-TB // P)
                    wid = [min(P, TB - s * P) for s in range(subs)]
                    groups = [[0, 1], [2], [3]] if first else [[s] for s in range(subs)]
                    for group in groups:
                        scs, prevq = {}, {}
                        for s in group:
                            scs[s] = scps_pool.tile(
                                [P, 512], FP32, tag="sc_ps", name=f"sc{s}"
                            )
                            prevq[s] = []

                        def dot(s, mc, h_sb):
                            nc.tensor.matmul(
                                scs[s][0 : wid[s], 0 : c.H],
                                h_sb[:, 0 : wid[s]],
                                w2_sb[:, mc, :],
                                start=(mc == 0),
                                stop=(mc == c.MC - 1),
                            )

                        for mc in range(c.MC):
                            for s in group:
                                w = wid[s]
                                h_ps = hps_pool.tile([P, 512], FP32, tag="h_ps")
                                wcol = (mc % QMC) * P
                                for dc in range(c.KD):
                                    nc.tensor.matmul(
                                        h_ps[:, 0:w],
                                        w1q[mc // QMC][:, dc, wcol : wcol + P],
                                        xT[:, dc, s * P : s * P + w],
                                        start=(dc == 0),
                                        stop=(dc == c.KD - 1),
                                    )
                                h_sb = h_pool.tile([P, P], BF16, tag="h_sb")
                                nc.scalar.activation(
                                    h_sb[:, 0:w], h_ps[:, 0:w], AFT.Tanh,
                                    bias=b1_sb[:, mc : mc + 1],
                                )
                                prevq[s].append((mc, h_sb))
                                if len(prevq[s]) > 2:
                                    dot(s, *prevq[s].pop(0))
                            if mc == 2:
                                flush(3)
                        for s in group:
                            while prevq[s]:
                                dot(s, *prevq[s].pop(0))
                        if pl_ps is None:
                            # full flush: the previous batch's pool/finish
                            # closures must be emitted before the bufs=1
                            # pl/z tiles rotate to this batch
                            flush()
                            pl_ps = plps_pool.tile([P, c.D], FP32, tag="plps")
                            z_ps = zps_pool.tile([P, 512], FP32, tag="zps")
                        for s in group:
                            w = wid[s]
                            e_blk = e_pool.tile([P, c.H], BF16, tag="e_blk")
                            nc.scalar.activation(
                                e_blk[0:w, :], scs[s][0:w, 0 : c.H],
                                AFT.Exp, bias=0.0,
                            )

                            def mk_pool(b, s, w, cn, ncv, e_blk, xnt, pl_ps, z_ps):
                                def go():
                                    nc.tensor.matmul(
                                        z_ps[0 : c.H, 0:1],
                                        e_blk[0:w, :],
                                        v_sb[0:w, b, cn : cn + 1],
                                        start=(cn == 0),
                                        stop=(cn == ncv - 1),
                                    )
                                    for hf in range(2):
                                        nc.tensor.matmul(
                                            pl_ps[0 : c.H, ts(hf, HD)],
                                            e_blk[0:w, :],
                                            xnt[0:w, s, ts(hf, HD)],
                                            start=(cn == 0),
                                            stop=(cn == ncv - 1),
                                        )
                                return go

                            pending.append(
                                mk_pool(b, s, w, cn0 + s, NCVb, e_blk, xnt,
                                        pl_ps, z_ps)
                            )
                    t0 += TB

                def mk_finish(b, pl_ps, z_ps):
                    def go():
                        rz = small_pool.tile([c.H, 1], FP32, tag="rz")
                        nc.vector.reciprocal(rz[:], z_ps[0 : c.H, 0:1])
                        r0 = b * 32
                        # halves on different engines (DVE + Act) in parallel
                        nc.vector.tensor_scalar_mul(
                            pooled_sb[r0 : r0 + c.H, 0:HD],
                            pl_ps[0 : c.H, 0:HD],
                            rz[:],
                        )
                        nc.scalar.activation(
                            pooled_sb[r0 : r0 + c.H, HD : c.D],
                            pl_ps[0 : c.H, HD : c.D],
                            AFT.Copy, scale=rz[:],
                        )
                    return go

                pending.append(mk_finish(b, pl_ps, z_ps))

            flush()
            # ---- pooled^T, then output projection y = pooled @ Wout + bout ----
            # tpp reuses sc-pool buffers: sc groups are long closed by now.
            # The two psum->sbuf copies run on different engines (DVE/Act).
            for grp in range(2):
                tpp = scps_pool.tile([P, 512], FP32, tag="sc_ps", name="tpp")
                for i in range(4):
                    dc = grp * 4 + i
                    nc.tensor.transpose(
                        tpp[:, i * P : (i + 1) * P],
                        pooled_sb[:, ts(dc, P)],
                        idR[:],
                    )
                eng = nc.vector.tensor_copy if grp == 0 else nc.scalar.copy
                eng(
                    poolT_sb[:, grp * 4 : (grp + 1) * 4, :],
                    tpp[:].rearrange("p (k r) -> p k r", r=P),
                )
            # y^T chunks [128 d_out, BL]: free dim = BL = 4, so the 256
            # matmuls cost the PE almost nothing (vs 64 x 512-free rows
            # the straight orientation would charge). Groups sequential
            # per d_out chunk -> one PSUM region suffices.
            yt_ps = zps_pool.tile([P, 512], FP32, tag="zps", name="yt_ps")
            for do in range(c.KD):
                for hd in range(c.H):
                    bcols = poolT_sb[:, :, :].rearrange(
                        "p k (b j) -> p k j b", j=32
                    )
                    ytcols = yt_ps[:, 0 : c.BL * c.KD].rearrange(
                        "p (b o) -> p o b", o=c.KD
                    )
                    for dc in range(c.KD):
                        nc.tensor.matmul(
                            ytcols[:, do, :],
                            wout_sb[:, hd * c.KD + dc, ts(do, P)],
                            bcols[:, dc, hd, :],
                            start=(hd == 0 and dc == 0),
                            stop=(hd == c.H - 1 and dc == c.KD - 1),
                        )
            # += bout (pre-laid as [P, KD*BL]), then a strided DMA writes
            # y directly in [BL, D] order - no transposes, no extra copy
            yt_sb = ysb_pool.tile([P, c.KD * c.BL], FP32)
            nc.vector.tensor_add(
                yt_sb[:], yt_ps[:, 0 : c.KD * c.BL], bout_sb[:]
            )
            nc.sync.dma_start(
                y_d.rearrange("b (o p) -> p b o", p=P),
                yt_sb[:].rearrange("p (b o) -> p b o", o=c.KD),
            )
    return nc


def make_in_maps(x, valid_mask, W1, b1, W2, b2, Wout, bout, n_cores, cfg,
                 order=None):
    """Host-side prep: compact valid tokens, shard over batch, cast/layout."""
    c = cfg
    bf16 = ml_dtypes.bfloat16
    B = x.shape[0]
    w1_l = np.ascontiguousarray(
        W1.transpose(1, 0, 2).reshape(c.KD, P, c.HF).astype(bf16)
    )
    w2f = W2.reshape(c.HF).astype(np.float32)
    w2_l = np.zeros((c.MC, P, c.H), np.float32)
    FC = c.MC // c.H  # f-chunks per head
    for mc in range(c.MC):
        w2_l[mc, :, mc // FC] = w2f[mc * P : (mc + 1) * P]
    w2_l = np.ascontiguousarray(w2_l.astype(bf16))
    b1_l = np.ascontiguousarray(b1.reshape(c.HF).astype(np.float32))
    wout_l = np.ascontiguousarray(Wout.reshape(c.KOUT, P, c.D).astype(bf16))
    # bout pre-laid for the y^T tail: bout_l[p, b*KD + do] = bout[do*128+p]
    bout_l = np.ascontiguousarray(
        np.tile(bout.astype(np.float32).reshape(c.KD, P).T, (1, c.BL))
    )
    # b2 is a per-row constant under the softmax -> it cancels; drop it.
    if order is None:
        order = np.arange(B)
    xc = np.zeros((B, c.NV, c.D), bf16)
    v = np.zeros((B, c.NCV * P), np.float32)
    for gb in range(B):
        idx = np.flatnonzero(valid_mask[gb])[: c.NV]
        xc[gb, : len(idx)] = x[gb, idx].astype(bf16)
        v[gb, : len(idx)] = 1.0
    v_l = np.ascontiguousarray(
        v.reshape(B, c.NCV, P).transpose(0, 2, 1).astype(bf16)
    )
    xt_all = np.ascontiguousarray(xc.transpose(0, 2, 1)).reshape(
        B, c.KD, P, c.NV
    )
    in_maps = []
    for core in range(n_cores):
        sel = [order[i * n_cores + core] for i in range(c.BL)]
        in_maps.append(
            {
                "xt": np.ascontiguousarray(xt_all[sel]),
                "xn": np.ascontiguousarray(xc[sel]),
                "v": np.ascontiguousarray(v_l[sel]),
                "w1": w1_l,
                "w2": w2_l,
                "b1": b1_l,
                "wout": wout_l,
                "bout": bout_l,
            }
        )
    return in_maps


_cached = {}
last_results = None


def kernel(x, valid_mask, W1, b1, W2, b2, Wout, bout, trace=False):
    global last_results
    x, valid_mask, W1, b1, W2, b2, Wout, bout = (
        np.asarray(a)
        for a in (x, valid_mask, W1, b1, W2, b2, Wout, bout)
    )
    B = x.shape[0]
    n_cores = 8
    NVS, order = choose_slots(valid_mask, n_cores, B // n_cores)
    cfg = Cfg(BL=B // n_cores, NVS=NVS)
    key = (B, NVS)
    if key not in _cached:
        nc = bacc.Bacc("TRN2", target_bir_lowering=False, debug=False)
        build_kernel(nc, cfg)
        nc.compile()
        _cached[key] = nc
    in_maps = make_in_maps(
        x, valid_mask, W1, b1, W2, b2, Wout, bout, n_cores, cfg, order
    )
    res = run_bass_kernel_spmd(
        _cached[key], in_maps, core_ids=list(range(n_cores)), trace=trace
    )
    last_results = res
    y = np.empty((B, x.shape[2]), np.float32)
    for core in range(n_cores):
        yc = np.asarray(res.results[core]["y"], np.float32)
        for i in range(cfg.BL):
            y[order[i * n_cores + core]] = yc[i]
    return y
